# revision 17
# baseline (speedup 1.0000x reference)
# MoE (top-2 of 8 experts, SwiGLU FFN) on 8 trn2 NeuronCores.
#
# Sharding strategy (expert-parallel, routing-aware):
#   The host computes the router (gate logits -> softmax -> top-2 ->
#   renormalized combine weights; ~0.01% of total FLOPs) and uses the
#   routing decision to SHARD the tokens: core e receives exactly the
#   tokens routed to expert e (padded to a common capacity C), plus
#   expert e's weights pre-laid-out in the exact order the device
#   consumes them.  Each core runs the fused SwiGLU FFN
#   ( silu(x@w1T) * (x@w3T) ) @ w2T over its token slice in bf16 with
#   fp32 PSUM accumulation.  The host then scatter-adds each expert's
#   output back with the combine weights.  This does 2/8 of the dense
#   all-experts compute (only routed tokens); the sharding hint's
#   "all-to-all dispatch/combine" is realized as the host-side
#   gather/scatter that sharding full inputs requires anyway.
#
# Device schedule (v2 — weight-load amortization):
#   The PE's per-matmul cost is stream(N cols) PLUS a serial ~130 ns
#   weight load (Ldweights).  v1 loaded weights per 416-col matmul
#   (3840 loads/iter -> ~0.5 ms of pure weight-load time).  v2 holds
#   each of the 768 distinct 128x128 weight tiles loaded exactly ONCE
#   per iteration and streams ALL C tokens through it (NB = ceil(C/512)
#   PSUM-bank-sized matmuls per load).  Tile emits one Ldweights per
#   matmul unconditionally, so a post-pass drops the back-to-back
#   duplicates (identical stationary AP, no intervening load) — the
#   Matmult ISA op is non-self-loading ("ldweights": false), so the
#   remaining matmuls legally reuse the loaded array state.
#
#   Stage A per i-chunk ic (h1 = x@w1T chunk, h3 = x@w3T chunk):
#     h1 chain:  for hc in 8: [ldw w1(hc,ic)] -> NB matmuls (all token
#                banks), accumulating into ps[b] over hc.
#     ACT silu:  ps[b] -> sl[b]  (overlaps tail of h1 chain)
#     h3 chain:  same banks, reused after ACT drains each.
#     DVE mul:   g[:,ic,:] = sl[b] * ps[b]   (bf16, stays on-chip)
#   Stage B per output h-chunk oc: for ic in 32: [ldw w2(oc,ic)] ->
#     NB matmuls accumulating ygT chunk over ic; DVE copy-out; DMA.
#
#   All weights are STREAMED from HBM each iteration (w1/w3 no longer
#   SBUF-resident) — this frees the ~130 KB/partition needed to hold
#   g for the full token capacity, which is what lets every weight
#   tile serve all C tokens in one load.  Weight DMA (~32 MB/iter at
#   358 GB/s ~ 90 us) hides entirely under ~700 us of PE work.
#   Outputs go out on the second HWDGE queue (ACT engine) so result
#   write-back never head-blocks the weight stream on the SP queue.

import math
import sys

import numpy as np

if "/opt/trn_rl_repo" not in sys.path:
    sys.path.insert(0, "/opt/trn_rl_repo")

import concourse.bass as bass
import concourse.mybir as mybir
import concourse.tile as tile
from concourse.bass_utils import run_bass_kernel_spmd

B, S, H, I, E, TOPK = 4, 2048, 1024, 4096, 8, 2
T = B * S
P = 128
HC = H // P   # 8 h-chunks (stage A contraction / stage B output)
IC = I // P   # 32 i-chunks
OC = H // P   # 8 output h-chunks
TB = 512      # PSUM bank = 512 fp32 -> max matmul free dim
MAX_SWEEP = 2176  # SBUF cap for g ([P, IC, sweep] bf16) + xg + bufs

_BF16 = mybir.dt.np(mybir.dt.bfloat16)

_PROGRAM_CACHE: dict = {}
_LAST_IN_MAPS = None

# This toolchain's walrus enforces small per-ISA-struct sync-wait budgets
# (a DVE TensorTensor takes ONE wait; the Tile-exit Drain takes one, etc.).
# Tile attaches as many waits as deps require, so we legalize post-hoc:
# hoist excess waits into standalone EventSemaphore instructions inserted
# immediately before the offending instruction on the same engine queue —
# semantically identical to carrying the wait on the instruction itself.
_WAIT_BUDGET: dict = {}
_DEFAULT_WAIT_BUDGET = 1


def _regroup_pe_matmuls(nc):
    """Re-sort (Ldweights, Matmult) pairs so same-weight matmuls are
    back-to-back, enabling Ldweights dedup.

    The Tile scheduler is greedy: when the highest-priority matmul isn't
    ready yet (its PSUM bank still draining), it runs ahead with a later
    chain, permanently staggering the per-bank accumulation chains and
    interleaving weights.  Since the PE engine executes its queue in
    FIFO order and same-bank matmuls are the only PE-internal ordering
    constraint (accumulation chains; cross-bank matmuls commute), we can
    statically re-sort each maximal run of (Ldweights, Matmult) pairs by
    weight tile, keeping the first-seen weight order.  Every bank chain
    visits weights in the same first-seen order, so a stable sort
    preserves all per-bank chain orders.  Semaphore waits move with
    their instructions; waits elsewhere that reference PE sem-inc
    thresholds ("first v matmuls done") are remapped to the moved
    instructions' new ranks.
    """
    import json as _json

    m = _json.loads(nc.to_json_bytes())

    # sem id -> ordered list of PE instructions carrying an update, plus
    # classification mirroring _coalesce_pe_sem_updates' candidate rules.
    from collections import defaultdict

    def pe_update_sems(insts):
        upd = defaultdict(list)
        for inst in insts:
            si = inst.get("sync_info") or {}
            for u in si.get("on_update") or []:
                upd[u["id"]].append(inst)
        return upd

    # Sems with sem-inc updates in >1 block have cross-block rank
    # semantics we must not touch.  (For_i's prime/reset add/sub-imm
    # updates in the loop skip/reset blocks are rank-neutral: they
    # add/remove the whole per-iteration total, so reordering the body's
    # incs under them is safe.)
    upd_blocks = defaultdict(set)
    for fn in m["functions"]:
        for bi, blk in enumerate(fn["blocks"]):
            for inst in blk["instructions"]:
                si = inst.get("sync_info") or {}
                for u in si.get("on_update") or []:
                    if u.get("update_mode") == "sem-inc":
                        upd_blocks[u["id"]].add((id(fn), bi))
    multi_block_sems = {s for s, bs in upd_blocks.items() if len(bs) > 1}

    n_moved = 0
    for fn in m["functions"]:
        for blk in fn["blocks"]:
            insts = blk["instructions"]
            pe_old = [i for i in insts if i["engine"] == "PE"]
            old_upd = pe_update_sems(pe_old)

            # Build runs over the PE substream (positions into insts).
            pe_pos = [k for k, i in enumerate(insts) if i["engine"] == "PE"]
            runs = []  # list of (list of positions) covering ldw/mm pairs
            cur = []
            for k in pe_pos:
                op = insts[k]["opcode"]
                if op in ("Ldweights", "Matmult"):
                    cur.append(k)
                else:
                    if cur:
                        runs.append(cur)
                    cur = []
            if cur:
                runs.append(cur)

            for run in runs:
                # pair up: expect strict [ldw, mm] alternation
                if len(run) < 4 or len(run) % 2:
                    continue
                ok = all(
                    insts[p]["opcode"] == ("Ldweights" if j % 2 == 0
                                           else "Matmult")
                    for j, p in enumerate(run))
                if not ok:
                    continue
                pairs = [(insts[run[2 * j]], insts[run[2 * j + 1]])
                         for j in range(len(run) // 2)]
                if any(u["id"] in multi_block_sems
                       for _, mm in pairs
                       for u in (mm.get("sync_info") or {}).get(
                           "on_update") or []):
                    continue
                wkey = {}
                order = []
                for ldw, mm in pairs:
                    k = _json.dumps([ldw["ins"], ldw.get("tile_position"),
                                     ldw.get("tile_size")], sort_keys=True)
                    if k not in wkey:
                        wkey[k] = len(wkey)
                    order.append(wkey[k])
                # verify each bank (psum memref) visits weights in
                # non-decreasing first-seen order; else skip (safety)
                bank_last = {}
                safe = True
                for (ldw, mm), o in zip(pairs, order):
                    bank = mm["outs"][0]["memref"]
                    if bank_last.get(bank, -1) > o:
                        safe = False
                        break
                    bank_last[bank] = o
                if not safe:
                    continue
                perm = sorted(range(len(pairs)), key=lambda j: (order[j], j))
                if perm == list(range(len(pairs))):
                    continue
                n_moved += sum(1 for j, p in enumerate(perm) if j != p)
                newpairs = [pairs[p] for p in perm]
                flat = [x for pr in newpairs for x in pr]
                for pos, inst in zip(run, flat):
                    insts[pos] = inst

            # Remap PE-sourced sem wait thresholds by update identity.
            pe_new = [i for i in insts if i["engine"] == "PE"]
            new_upd = pe_update_sems(pe_new)
            remaps = {}
            for sid, old_list in old_upd.items():
                new_list = new_upd.get(sid, [])
                if [id(x) for x in old_list] == [id(x) for x in new_list]:
                    continue
                assert {id(x) for x in old_list} == {id(x) for x in new_list}
                rank = {id(inst): r + 1 for r, inst in enumerate(new_list)}
                remaps[sid] = [rank[id(inst)] for inst in old_list]
            if remaps:
                for fn2 in m["functions"]:
                    for blk2 in fn2["blocks"]:
                        for inst in blk2["instructions"]:
                            si = inst.get("sync_info") or {}
                            for w in si.get("on_wait") or []:
                                sid = w["id"]
                                if sid in remaps and isinstance(
                                        w.get("wait_value"), int):
                                    v = w["wait_value"]
                                    if 1 <= v <= len(remaps[sid]):
                                        w["wait_value"] = remaps[sid][v - 1]

    data = _json.dumps(m).encode()
    nc.to_json_bytes = lambda: data
    return n_moved


def _dedup_ldweights(nc):
    """Drop Ldweights that reload the PE array with weights already there.

    Tile emits one Ldweights before every Matmult.  When consecutive
    matmuls use the IDENTICAL stationary AP (same buffer, offset, access
    pattern, dtype, tile position/size), the repeats are pure overhead:
    the Matmult ISA instruction is non-self-loading ("ldweights": false
    in the emitted stream) and PE array weight state persists across
    matmuls.  Keep the first load, drop the rest; any sync waits the
    dropped loads carried are preserved as standalone EventSemaphore
    instructions at the same queue position.
    """
    import json as _json

    m = _json.loads(nc.to_json_bytes())
    n_drop = 0
    n_hoist = 0
    for fn in m["functions"]:
        for blk in fn["blocks"]:
            out = []
            last_ldw = None
            pending_waits = []
            for inst in blk["instructions"]:
                if inst["engine"] != "PE":
                    out.append(inst)
                    continue
                if inst["opcode"] == "Ldweights":
                    key = _json.dumps(
                        [inst["ins"], inst.get("tile_position"),
                         inst.get("tile_size"), inst.get("perf_mode"),
                         inst.get("is_transpose")],
                        sort_keys=True)
                    si = inst.get("sync_info") or {}
                    if last_ldw == key and not (si.get("on_update") or []):
                        n_drop += 1
                        pending_waits.extend(si.get("on_wait") or [])
                        continue
                    last_ldw = key
                elif inst["opcode"] == "Matmult":
                    pass  # non-self-loading; array state persists
                else:
                    # Drain/branch/etc: conservatively forget the loaded
                    # weights so we never dedupe across control flow.
                    last_ldw = None
                if pending_waits:
                    for w in pending_waits:
                        n_hoist += 1
                        out.append({
                            "debug": inst.get("debug", 0),
                            "engine": "PE",
                            "ins": [], "outs": [],
                            "name": f"I-ldwde-{n_hoist}",
                            "opcode": "EventSemaphore",
                            "sync_info": {"on_update": [], "on_wait": [w]},
                        })
                    pending_waits = []
                out.append(inst)
            assert not pending_waits
            blk["instructions"] = out
    data = _json.dumps(m).encode()
    nc.to_json_bytes = lambda: data
    return n_drop


def _coalesce_pe_sem_updates(nc):
    """Drop PE semaphore increments nobody waits on.

    Every matmul carries a sem-inc, but each inc is a serialized EVT_SEM
    register write (~26 ns) on the PE NX path.  Since PE instructions
    complete in program order, a wait for "first k matmuls done" is
    equivalently a wait on the k-th *kept* increment.  So: keep an
    increment only at stream positions some wait references (plus the
    final one, preserving the end-of-block total), and renumber all wait
    thresholds to rank-among-kept.  Ordering semantics are unchanged by
    construction.
    """
    import json as _json
    from collections import defaultdict

    m = _json.loads(nc.to_json_bytes())
    fns = m["functions"]

    upd_src = defaultdict(set)    # sem id -> {(engine, block idx)} (incs)
    upd_mode = defaultdict(set)   # sem id -> {(mode, value)} (incs)
    aux_upds = defaultdict(list)  # sem id -> non-inc update dicts
    wait_mode = defaultdict(set)
    wait_vals = defaultdict(set)
    for fn in fns:
        for bi, blk in enumerate(fn["blocks"]):
            for inst in blk["instructions"]:
                si = inst.get("sync_info") or {}
                for u in si.get("on_update") or []:
                    if u.get("update_mode") == "sem-inc":
                        upd_src[u["id"]].add((inst["engine"], bi))
                        upd_mode[u["id"]].add((u.get("update_mode"),
                                               u.get("update_value")))
                    else:
                        aux_upds[u["id"]].append(u)
                for w in si.get("on_wait") or []:
                    wait_mode[w["id"]].add(w.get("wait_mode"))
                    wait_vals[w["id"]].add(w.get("wait_value"))

    # Sems fed by PE sem-inc(1) in a single block: their cumulative value
    # == position in the PE instruction stream.  For_i-lowered programs
    # additionally prime/reset the stream sem with add/sub-imm of the
    # whole per-iteration total in the loop skip/reset blocks — those are
    # rank-neutral and get rewritten to the new kept-inc total below.
    cands = [s for s in upd_src
             if len(upd_src[s]) == 1
             and next(iter(upd_src[s]))[0] == "PE"
             and upd_mode[s] == {("sem-inc", 1)}
             and wait_mode.get(s, set()) <= {"sem-ge-imm"}
             and all(u.get("update_mode") in ("sem-add-imm", "sem-sub-imm")
                     for u in aux_upds.get(s, []))]

    n_dropped = 0
    for sid in cands:
        updates = []
        for fn in fns:
            for blk in fn["blocks"]:
                for inst in blk["instructions"]:
                    if inst["engine"] != "PE":
                        continue
                    si = inst.get("sync_info") or {}
                    for u in si.get("on_update") or []:
                        if u["id"] == sid and u.get("update_mode") == "sem-inc":
                            updates.append((inst, u))
        n = len(updates)
        if not n:
            continue
        if any(not isinstance(u.get("update_value"), int)
               or u["update_value"] != n for u in aux_upds.get(sid, [])):
            continue  # prime/reset amount isn't the body total; skip
        vals = wait_vals.get(sid, set())
        if any(not isinstance(v, int) or v < 0 or v > n for v in vals):
            continue  # unexpected threshold; leave this sem alone
        keep = set(v for v in vals if v >= 1)
        keep.add(n)
        remap = {}
        rank = 0
        for cum in range(1, n + 1):
            if cum in keep:
                rank += 1
                remap[cum] = rank
        for cum, (inst, u) in enumerate(updates, 1):
            if cum not in keep:
                si = inst["sync_info"]
                si["on_update"] = [x for x in si["on_update"] if x is not u]
                n_dropped += 1
        n_kept = len(keep)
        for u in aux_upds.get(sid, []):
            u["update_value"] = n_kept
        for fn in fns:
            for blk in fn["blocks"]:
                for inst in blk["instructions"]:
                    si = inst.get("sync_info") or {}
                    for w in si.get("on_wait") or []:
                        if w["id"] == sid and w["wait_value"] >= 1:
                            w["wait_value"] = remap[w["wait_value"]]

    data = _json.dumps(m).encode()
    nc.to_json_bytes = lambda: data
    return n_dropped


def _legalize_sync_waits(nc):
    import json as _json

    m = _json.loads(nc.to_json_bytes())
    n_new = 0
    for fn in m["functions"]:
        for blk in fn["blocks"]:
            out = []
            for inst in blk["instructions"]:
                si = inst.get("sync_info")
                waits = (si or {}).get("on_wait") or []
                budget = _WAIT_BUDGET.get(inst.get("opcode"),
                                          _DEFAULT_WAIT_BUDGET)
                if len(waits) > budget:
                    for w in waits[:-budget]:
                        n_new += 1
                        out.append({
                            "debug": inst.get("debug", 0),
                            "engine": inst["engine"],
                            "ins": [],
                            "outs": [],
                            "name": f"I-legw-{n_new}",
                            "opcode": "EventSemaphore",
                            "sync_info": {"on_update": [], "on_wait": [w]},
                        })
                    si["on_wait"] = waits[-budget:]
                out.append(inst)
            blk["instructions"] = out
    data = _json.dumps(m).encode()
    nc.to_json_bytes = lambda: data  # shadow for bass2jax/compile paths
    return n_new


def _postprocess(nc):
    """All IR post-passes, in order."""
    r = _regroup_pe_matmuls(nc)
    a = _dedup_ldweights(nc)
    b = _coalesce_pe_sem_updates(nc)
    c = _legalize_sync_waits(nc)
    return r, a, b, c


def _sweep_sizes(C):
    """Split C into sweeps of <= MAX_SWEEP tokens (each sweep re-streams
    the full weight set), and each sweep into <= ceil/512 PSUM-bank
    chunks of near-equal multiples of 32."""
    n_sweeps = max(1, math.ceil(C / MAX_SWEEP))
    per, extra = divmod(C // 32, n_sweeps)
    sweeps = [(per + (1 if i < extra else 0)) * 32 for i in range(n_sweeps)]
    out = []
    for sw in sweeps:
        nb = max(1, math.ceil(sw / TB))
        per_b, extra_b = divmod(sw // 32, nb)
        out.append([(per_b + (1 if i < extra_b else 0)) * 32
                    for i in range(nb)])
    return out


def _build_program(C: int, repeat: int = 1):
    """One SPMD Bass program: fused SwiGLU FFN over [H, C] tokens.

    repeat > 1 wraps the body in a hardware For_i loop — used by the
    benchmark harness to amortize host/axon dispatch overhead out of
    wall-clock timings.
    """
    dt_in = mybir.dt.bfloat16
    f32 = mybir.dt.float32
    nc = bass.Bass()
    sweeps = _sweep_sizes(C)
    max_sw = max(sum(s) for s in sweeps)
    max_nb = max(len(s) for s in sweeps)

    xgT = nc.dram_tensor("xgT", [H, C], dt_in, kind="ExternalInput")
    w13s = nc.dram_tensor("w13s", [P, IC * 2 * HC * P], dt_in,
                          kind="ExternalInput")
    w2s = nc.dram_tensor("w2s", [P, OC * IC * P], dt_in,
                         kind="ExternalInput")
    ygT = nc.dram_tensor("ygT", [H, C], f32, kind="ExternalOutput")

    xgT_r = xgT.rearrange("(hc p) c -> p hc c", p=P)
    ygT_r = ygT.rearrange("(oc p) c -> p oc c", p=P)

    with tile.TileContext(nc) as tc:
        with (
            tc.tile_pool(name="xg", bufs=1) as xpool,
            tc.tile_pool(name="g", bufs=1) as gpool,
            tc.tile_pool(name="w13", bufs=2) as wpool,
            tc.tile_pool(name="w2", bufs=2) as w2pool,
            tc.tile_pool(name="sl", bufs=1) as spool,
            tc.tile_pool(name="ot", bufs=3) as opool,
            tc.tile_pool(name="ps", bufs=1, space="PSUM") as pspool,
        ):
            # Token activations are loaded OUTSIDE the repeat loop: the
            # For_i lowering places a full all-engine barrier between
            # iterations, so an in-loop xg load would stall the PE ~12us
            # at every iteration start.  xg is read-only in the body; in
            # benchmark (repeat) mode iterations simply re-read it.
            xg_tiles = []
            off = 0
            for si_, s_sizes in enumerate(sweeps):
                SW = sum(s_sizes)
                xg = xpool.tile([P, HC, max_sw], dt_in, tag=f"xg{si_}",
                                name=f"xg{si_}")
                nc.sync.dma_start(xg[:, :, :SW], xgT_r[:, :, off:off + SW])
                xg_tiles.append(xg)
                off += SW

            from contextlib import nullcontext
            rep_ctx = tc.For_i(0, repeat, 1) if repeat > 1 else nullcontext()
            with rep_ctx:
                off = 0
                for si_, s_sizes in enumerate(sweeps):
                    _sweep(nc, tc, dt_in, f32, xg_tiles[si_], ygT_r,
                           w13s, w2s, off, s_sizes, max_sw, max_nb,
                           gpool, wpool, w2pool, spool, opool, pspool)
                    off += sum(s_sizes)
    return nc


def _sweep(nc, tc, dt_in, f32, xg, ygT_r, w13s, w2s, off, sizes,
           max_sw, max_nb, gpool, wpool, w2pool, spool, opool, pspool):
    SW = sum(sizes)
    NB = len(sizes)
    offs = [sum(sizes[:i]) for i in range(NB)]
    silu = mybir.ActivationFunctionType.Silu

    g = gpool.tile([P, IC, max_sw], dt_in, tag="g")

    ps = [None] * NB
    # Stage A: g = silu(x@w1T) * (x@w3T), one i-chunk at a time.
    for ic in range(IC):
        # One stream chunk per ic: the 16 stage-A weight tiles
        # (w1 hc0-7 then w3 hc0-7), DMA'd as one contiguous 4KB/row copy.
        wch = wpool.tile([P, 2 * HC * P], dt_in, tag="w13")
        c0 = ic * 2 * HC * P
        nc.sync.dma_start(wch[:], w13s[:, c0:c0 + 2 * HC * P])

        # h1 chains: each weight tile loaded once, streamed over all NB
        # token banks (the dedup post-pass keeps a single Ldweights).
        for hc in range(HC):
            for b in range(NB):
                ps_b = pspool.tile([P, TB], f32, tag=f"ps{b}",
                                   name=f"ps{b}") if hc == 0 else ps[b]
                if hc == 0:
                    ps[b] = ps_b
                nc.tensor.matmul(ps_b[:, :sizes[b]],
                                 wch[:, hc * P:(hc + 1) * P],
                                 xg[:, hc, offs[b]:offs[b] + sizes[b]],
                                 start=(hc == 0), stop=(hc == HC - 1))
        sl = [None] * NB
        for b in range(NB):
            sl[b] = spool.tile([P, TB], f32, tag=f"sl{b}", name=f"sl{b}")
            nc.scalar.activation(sl[b][:, :sizes[b]], ps[b][:, :sizes[b]],
                                 silu)
        # h3 chains: same PSUM banks, reused bank-by-bank as ACT drains.
        for hc in range(HC):
            for b in range(NB):
                ps_b = pspool.tile([P, TB], f32, tag=f"ps{b}",
                                   name=f"ps{b}") if hc == 0 else ps[b]
                if hc == 0:
                    ps[b] = ps_b
                nc.tensor.matmul(ps_b[:, :sizes[b]],
                                 wch[:, (HC + hc) * P:(HC + hc + 1) * P],
                                 xg[:, hc, offs[b]:offs[b] + sizes[b]],
                                 start=(hc == 0), stop=(hc == HC - 1))
        for b in range(NB):
            nc.vector.tensor_mul(
                out=g[:, ic, offs[b]:offs[b] + sizes[b]],
                in0=sl[b][:, :sizes[b]], in1=ps[b][:, :sizes[b]])

    # Stage B: ygT[oc] = sum_ic w2(oc,ic).T @ g[ic], one output h-chunk
    # at a time; 16-tile weight stream chunks (2 per oc).
    W2CH = 16
    for oc in range(OC):
        for half in range(IC // W2CH):
            wch2 = w2pool.tile([P, W2CH * P], dt_in, tag="w2")
            c0 = (oc * IC + half * W2CH) * P
            nc.sync.dma_start(wch2[:], w2s[:, c0:c0 + W2CH * P])
            for t in range(W2CH):
                ic = half * W2CH + t
                for b in range(NB):
                    ps_b = pspool.tile([P, TB], f32, tag=f"ps{b}",
                                       name=f"ps{b}") if ic == 0 else ps[b]
                    if ic == 0:
                        ps[b] = ps_b
                    nc.tensor.matmul(ps_b[:, :sizes[b]],
                                     wch2[:, t * P:(t + 1) * P],
                                     g[:, ic, offs[b]:offs[b] + sizes[b]],
                                     start=(ic == 0), stop=(ic == IC - 1))
        for b in range(NB):
            ot = opool.tile([P, TB], f32, tag="ot")
            nc.vector.tensor_copy(ot[:, :sizes[b]], ps[b][:, :sizes[b]])
            # Second HWDGE queue (ACT engine): result write-back must not
            # head-block the weight stream on the SP queue.
            nc.scalar.dma_start(
                ygT_r[:, oc, off + offs[b]:off + offs[b] + sizes[b]],
                ot[:, :sizes[b]])


def _route(xt: np.ndarray, Wg: np.ndarray):
    """Host router: softmax over gate logits, top-2, renormalized weights."""
    logits = xt @ Wg.T.astype(np.float32)                       # [T, E]
    logits = logits - logits.max(axis=1, keepdims=True)
    p = np.exp(logits, dtype=np.float32)
    p /= p.sum(axis=1, keepdims=True)
    r = np.arange(T)
    top1 = p.argmax(axis=1)
    p2 = p.copy()
    p2[r, top1] = -1.0
    top2 = p2.argmax(axis=1)
    v1 = p[r, top1]
    v2 = p[r, top2]
    den = v1 + v2
    v1 = v1 / den
    v2 = v2 / den
    idxs, wts = [], []
    for e in range(E):
        m1 = top1 == e
        sel = np.nonzero(m1 | (top2 == e))[0]
        idxs.append(sel)
        wts.append(np.where(m1, v1, v2)[sel].astype(np.float32))
    return idxs, wts


def _pack_w13(w1e: np.ndarray, w3e: np.ndarray) -> np.ndarray:
    """Stage-A weight stream: [P, IC*2*HC*P] bf16, tiles in consumption
    order (ic-major; w1 hc0-7 then w3 hc0-7).  Tile (j,hc,ic) is
    w{1,3}T[hc*P:(hc+1)*P, ic*P:(ic+1)*P] (partition dim = h rows)."""
    a1 = w1e.reshape(IC, P, HC, P).transpose(3, 0, 2, 1)  # [p, ic, hc, ii]
    a3 = w3e.reshape(IC, P, HC, P).transpose(3, 0, 2, 1)
    w = np.stack([a1, a3], axis=2)                        # [p, ic, j, hc, ii]
    return np.ascontiguousarray(w.reshape(P, IC * 2 * HC * P)).astype(_BF16)


def _pack_w2(w2e: np.ndarray) -> np.ndarray:
    """Stage-B weight stream: [P, OC*IC*P] bf16, tiles in consumption
    order (oc-major, ic inner).  Tile (oc,ic) is
    w2T[ic*P:(ic+1)*P, oc*P:(oc+1)*P] (partition dim = i rows)."""
    a2 = w2e.reshape(OC, P, IC, P).transpose(3, 0, 2, 1)  # [p, oc, ic, ii]
    return np.ascontiguousarray(a2.reshape(P, OC * IC * P)).astype(_BF16)


def _run_with_retry(nc, in_maps, core_ids, attempts=4):
    """The axon-tunneled NeuronCores intermittently report
    NRT_EXEC_UNIT_UNRECOVERABLE right after a previous process used them;
    a fresh PJRT client after a cool-down recovers.  Retry transparently."""
    import time as _time

    for k in range(attempts):
        try:
            return run_bass_kernel_spmd(nc, in_maps, core_ids).results
        except Exception:
            if k == attempts - 1:
                raise
            try:
                import jax.extend as _jex
                _jex.backend.clear_backends()
            except Exception:
                pass
            _time.sleep(60 * (k + 1))


def kernel(x, Wg, w1, w3, w2):
    xt = np.ascontiguousarray(np.asarray(x, dtype=np.float32).reshape(T, H))
    idxs, wts = _route(xt, np.asarray(Wg, dtype=np.float32))
    counts = [len(ix) for ix in idxs]
    C = max(32, ((max(counts) + 31) // 32) * 32)

    key = C
    if key in _PROGRAM_CACHE:
        nc = _PROGRAM_CACHE[key]
    else:
        nc = _build_program(C)
        _postprocess(nc)
        _PROGRAM_CACHE[key] = nc

    # Per-core inputs: gathered tokens + this expert's weights, all
    # pre-laid-out on the host so every device DMA is a contiguous read.
    w1_ = np.asarray(w1)
    w3_ = np.asarray(w3)
    w2_ = np.asarray(w2)
    in_maps = []
    for e in range(E):
        xg = np.zeros((H, C), dtype=_BF16)
        xg[:, :counts[e]] = xt[idxs[e]].T.astype(_BF16)
        in_maps.append({
            "xgT": xg,
            "w13s": _pack_w13(w1_[e], w3_[e]),
            "w2s": _pack_w2(w2_[e]),
        })

    global _LAST_IN_MAPS
    _LAST_IN_MAPS = in_maps
    results = _run_with_retry(nc, in_maps, list(range(E)))

    out = np.zeros((T, H), dtype=np.float32)
    for e in range(E):
        yg = results[e]["ygT"]                       # [H, C] fp32
        out[idxs[e]] += wts[e][:, None] * yg[:, :counts[e]].T
    return out.reshape(B, S, H)
